# revision 7
# baseline (speedup 1.0000x reference)
"""Causal single-head attention (B=4, S=2048, D=1024) on 8 Trainium2 cores.

Sharding: 8 cores = (batch b, stripe-set eta). Core (b, eta) owns four
interleaved key stripes of 256 rows at global offsets 512k + 256*eta
(k = 0..3) of batch b, stored locally stripe-major (local key
ell in [256k, 256k+256) -> global 512k + 256*eta + ell%256). Queries are
fed "aligned" with base beta = 256*eta: query col c corresponds to global
row beta + c. Then the causal condition for local key tile kt vs query
chunk rc is c >= 512*(kt//2) + 128*(kt%2) + x — identical on every core,
so one SPMD program serves both stripe sets with a purely compile-time
block mask, and score blocks with kt >= 2*(rc+1) are skipped outright
(20 of 32 blocks kept vs 28 for a half-split). Cols past the sequence end
(eta=1, c >= 1792) compute junk that the host discards.

Softmax uses no max-subtraction (logits are O(1) for this problem:
|score/32| < ~4), so per-core partials are just num = exp(S)·V and
l = sum(exp(S)); the host merges halves with num/den addition and one
divide. This is mathematically identical to the reference softmax.

On-chip layout (all matmul operands bf16, fp32 PSUM accumulation):
  xt  = x_core^T              [D=1024, NQ=2048]  (keys are cols 0:1024)
  QT[o,r], KT[o,j] transposed; V[j,d] natural — chosen so every matmul
  is a plain lhsT.T @ rhs with no on-chip transposes:
    KT = wkT.T @ xt,  QT = wqT.T @ xt,  V = xt.T @ wvT
    ST = KT.T @ QT    (scores transposed: partition=key, free=query)
    PT = exp(ST/32) masked to r>=j;  OT = V.T @ PT;  l = 1s.T @ PT
"""

import sys

sys.path.insert(0, "/opt/trn_rl_repo")

from contextlib import ExitStack

import ml_dtypes
import numpy as np

import concourse.bass as bass  # noqa: F401  (engine types resolve via bacc)
import concourse.mybir as mybir
import concourse.tile as tile
from concourse import bacc, bass_utils
from concourse.bass import ts

BF16 = mybir.dt.bfloat16
FP8 = mybir.dt.float8e4
F32 = mybir.dt.float32

P = 128            # partitions
D = 1024           # model dim (d_in == d_out)
NQ = 2048          # query slots per core
NK = 1024          # keys per core
RC = 512           # query-chunk (matmul moving-dim) size
N_RC = NQ // RC    # 4
N_KT = NK // P     # 8 key tiles
N_IB = D // P      # 8 contraction blocks
SCALE = 1.0 / 32.0 # 1/sqrt(D)

N_CORES = 8
B, S = 4, 2048
STRIPE = 256


def _kept_kts(rc):
    # key tile kt (stripe k = kt//2) is visible to query chunk rc iff some
    # col c in [rc*512, rc*512+512) has c >= 512*(kt//2) + 128*(kt%2).
    return [kt for kt in range(N_KT) if kt < 2 * (rc + 1)]


def _mask_base(rc, kt):
    # keep when  y + 512*rc >= x + 512*(kt//2) + 128*(kt%2)
    return RC * rc - RC * (kt // 2) - P * (kt % 2)


def _emit(nc, tc, xt, wqt, wkt, wvt, ot, ls):
    with ExitStack() as ctx:
        sb = ctx.enter_context(tc.tile_pool(name="sb", bufs=1))
        pts = ctx.enter_context(tc.tile_pool(name="pts", bufs=1))
        outp = ctx.enter_context(tc.tile_pool(name="outp", bufs=3))
        ps = ctx.enter_context(tc.tile_pool(name="ps", bufs=6, space="PSUM"))
        psl = ctx.enter_context(tc.tile_pool(name="psl", bufs=2, space="PSUM"))

        ones = sb.tile([P, 1], BF16, tag="ones", name="ones")
        nc.vector.memset(ones, 1.0)

        # ---- input loads ----
        # Emission order = consumption order, so the first KT matmul can
        # start ~1.5us in (needs only wk[0] + xt[0] low half) instead of
        # stalling on the full 10MB input load.
        xt_sb = [sb.tile([P, NQ], BF16, tag=f"xt{i}", name=f"xt{i}")
                 for i in range(N_IB)]
        w_sb = {nm: [sb.tile([P, D], BF16, tag=f"{nm}{i}", name=f"{nm}{i}")
                     for i in range(N_IB)]
                for nm in ("wk", "wq", "wv")}
        xkv_sb = [sb.tile([P, NK], BF16, tag=f"xkv{i}", name=f"xkv{i}")
                  for i in range(N_IB)]
        for i in range(N_IB):
            nc.sync.dma_start(out=w_sb["wk"][i], in_=wkt[ts(i, P), :])
            # gather the 4 key stripes (cols 512k..512k+256 of xt) into a
            # dense [128, 1024] stripe-major kv tile
            nc.sync.dma_start(
                out=xkv_sb[i].rearrange("p (k c) -> p k c", c=256),
                in_=xt[ts(i, P), :].rearrange("p (k c) -> p k c", c=512)[:, :, 0:256])
        for i in range(N_IB):
            nc.sync.dma_start(out=w_sb["wv"][i], in_=wvt[ts(i, P), :])
        for i in range(N_IB):
            nc.sync.dma_start(out=w_sb["wq"][i], in_=wqt[ts(i, P), :])
        for i in range(N_IB):
            nc.sync.dma_start(out=xt_sb[i][:, 0:NK], in_=xt[ts(i, P), 0:NK])
        for i in range(N_IB):
            nc.sync.dma_start(out=xt_sb[i][:, NK:NQ], in_=xt[ts(i, P), NK:NQ])

        # ---- projections ----
        # i-major emission in batches of 4 PSUM groups: each arriving DMA
        # block immediately feeds 4 matmuls, and consecutive matmuls with
        # the same stationary operand sit adjacent in the PE stream.
        def proj_phase(groups, dst, lhs_of, rhs_of):
            for gb in range(0, len(groups), 4):
                batch = groups[gb:gb + 4]
                accs = [ps.tile([P, RC], F32, tag="mm", name="acc_p")
                        for _ in batch]
                for i in range(N_IB):
                    for a, g in zip(accs, batch):
                        nc.tensor.matmul(a, lhsT=lhs_of(i, g),
                                         rhs=rhs_of(i, g),
                                         start=(i == 0), stop=(i == N_IB - 1))
                for a, g in zip(accs, batch):
                    nc.vector.tensor_copy(dst(g), a)

        kt_sb = [sb.tile([P, NK], BF16, tag=f"ktk{o}", name=f"ktk{o}")
                 for o in range(N_IB)]
        proj_phase([(o, jc) for o in range(N_IB) for jc in range(NK // RC)],
                   dst=lambda g: kt_sb[g[0]][:, ts(g[1], RC)],
                   lhs_of=lambda i, g: w_sb["wk"][i][:, ts(g[0], P)],
                   rhs_of=lambda i, g: xkv_sb[i][:, ts(g[1], RC)])

        v_sb = [sb.tile([P, D], BF16, tag=f"vj{j}", name=f"vj{j}")
                for j in range(N_KT)]
        proj_phase([(j, dc) for j in range(N_KT) for dc in range(D // RC)],
                   dst=lambda g: v_sb[g[0]][:, ts(g[1], RC)],
                   lhs_of=lambda i, g: xkv_sb[i][:, ts(g[0], P)],
                   rhs_of=lambda i, g: w_sb["wv"][i][:, ts(g[1], RC)])

        qt_sb = [sb.tile([P, NQ], BF16, tag=f"qtq{o}", name=f"qtq{o}")
                 for o in range(N_IB)]
        proj_phase([(o, rc) for o in range(N_IB) for rc in range(N_RC)],
                   dst=lambda g: qt_sb[g[0]][:, ts(g[1], RC)],
                   lhs_of=lambda i, g: w_sb["wq"][i][:, ts(g[0], P)],
                   rhs_of=lambda i, g: xt_sb[i][:, ts(g[1], RC)])

        # ---- attention ----
        # ST is emitted kt-major so the stationary K^T block is reused by
        # consecutive matmuls; PV uses P^T sub-blocks as the stationary
        # operand (V moving), which makes the row-sum l an extra N=1 matmul
        # on an already-loaded stationary and yields output in natural
        # [query, d] orientation.
        pt_tiles = {}

        def kept_rcs(kt):
            return [rc for rc in range(N_RC) if kt in _kept_kts(rc)]

        def emit_st(kt):
            rcs = kept_rcs(kt)
            accs = {rc: ps.tile([P, RC], F32, tag="mm", name="acc_st")
                    for rc in rcs}
            for o in range(N_IB):
                for rc in rcs:
                    nc.tensor.matmul(accs[rc],
                                     lhsT=kt_sb[o][:, ts(kt, P)],
                                     rhs=qt_sb[o][:, ts(rc, RC)],
                                     start=(o == 0), stop=(o == N_IB - 1))
            for rc in rcs:
                pt = pts.tile([P, RC], BF16, tag=f"pt{kt}_{rc}",
                              name=f"pt{kt}_{rc}")
                nc.scalar.activation(pt, accs[rc],
                                     mybir.ActivationFunctionType.Exp,
                                     scale=SCALE)
                base = _mask_base(rc, kt)
                if base < P - 1:  # tile straddles the causal diagonal
                    nc.gpsimd.affine_select(
                        out=pt, in_=pt,
                        compare_op=mybir.AluOpType.is_ge, fill=0.0,
                        base=base, channel_multiplier=-1, pattern=[[1, RC]])
                pt_tiles[(kt, rc)] = pt

        l_sb = sb.tile([P, N_RC * 4], F32, tag="lsb", name="lsb")

        def emit_pv(rc):
            kts = _kept_kts(rc)
            last = len(kts) - 1
            for rsub in range(RC // P):
                pos = [ps.tile([P, RC], F32, tag="mm", name="acc_pv")
                       for _ in range(D // RC)]
                pl = psl.tile([P, 1], F32, tag="lp", name="lp")
                for n, kt in enumerate(kts):
                    lhs = pt_tiles[(kt, rc)][:, ts(rsub, P)]
                    for dc, po in enumerate(pos):
                        nc.tensor.matmul(po, lhsT=lhs,
                                         rhs=v_sb[kt][:, ts(dc, RC)],
                                         start=(n == 0), stop=(n == last))
                    nc.tensor.matmul(pl, lhsT=lhs, rhs=ones,
                                     start=(n == 0), stop=(n == last))
                row = rc * RC + rsub * P
                for dc, po in enumerate(pos):
                    o_sb = outp.tile([P, RC], F32, tag="osb", name="osb")
                    nc.vector.tensor_copy(o_sb, po)
                    nc.sync.dma_start(out=ot[row:row + P, ts(dc, RC)],
                                      in_=o_sb)
                nc.vector.tensor_copy(l_sb[:, rc * 4 + rsub:rc * 4 + rsub + 1],
                                      pl)

        # software-pipelined emission: PV(rc) right after its last key tile
        emit_st(0)
        emit_st(1)
        emit_pv(0)
        emit_st(2)
        emit_st(3)
        emit_pv(1)
        emit_st(4)
        emit_st(5)
        emit_pv(2)
        emit_st(6)
        emit_st(7)
        emit_pv(3)
        nc.sync.dma_start(out=ls, in_=l_sb)


_NC_CACHE = {}


def _get_nc():
    if "nc" not in _NC_CACHE:
        nc = bacc.Bacc("TRN2", target_bir_lowering=False, debug=False,
                       enable_asserts=False, num_devices=N_CORES)
        xt = nc.dram_tensor("xt", [D, NQ], BF16, kind="ExternalInput").ap()
        wqt = nc.dram_tensor("wqt", [D, D], BF16, kind="ExternalInput").ap()
        wkt = nc.dram_tensor("wkt", [D, D], BF16, kind="ExternalInput").ap()
        wvt = nc.dram_tensor("wvt", [D, D], BF16, kind="ExternalInput").ap()
        ot = nc.dram_tensor("ot", [NQ, D], F32, kind="ExternalOutput").ap()
        ls = nc.dram_tensor("ls", [P, N_RC * 4], F32, kind="ExternalOutput").ap()
        with tile.TileContext(nc) as tc:
            _emit(nc, tc, xt, wqt, wkt, wvt, ot, ls)
        nc.compile()
        _NC_CACHE["nc"] = nc
    return _NC_CACHE["nc"]


def make_in_maps(x, w_query, w_key, w_value):
    bf = ml_dtypes.bfloat16
    wqt = np.ascontiguousarray(np.asarray(w_query).T).astype(bf)
    wkt = np.ascontiguousarray(np.asarray(w_key).T).astype(bf)
    wvt = np.ascontiguousarray(np.asarray(w_value).T).astype(bf)
    in_maps = []
    for c in range(N_CORES):
        b, eta = c // 2, c % 2
        rows = (np.arange(NQ) + eta * STRIPE) % S  # cols past S wrap to junk
        xt_np = np.ascontiguousarray(np.asarray(x)[b, rows].T).astype(bf)
        in_maps.append({"xt": xt_np, "wqt": wqt, "wkt": wkt, "wvt": wvt})
    return in_maps


def merge_outputs(results):
    num = np.zeros((B, S, D), np.float32)
    den = np.zeros((B, S), np.float32)
    for c in range(N_CORES):
        b, eta = c // 2, c % 2
        otc = np.asarray(results[c]["ot"])   # [NQ, D]
        # ls[p, col] holds l for query col c = col*128 + p
        lc = np.asarray(results[c]["ls"]).T.reshape(NQ)
        beta = eta * STRIPE
        nvalid = S - beta
        num[b, beta:] += otc[:nvalid]
        den[b, beta:] += lc[:nvalid]
    return (num / den[:, :, None]).astype(np.float32)


def kernel(x, w_query, w_key, w_value, _trace=False):
    nc = _get_nc()
    in_maps = make_in_maps(x, w_query, w_key, w_value)
    res = bass_utils.run_bass_kernel_spmd(
        nc, in_maps, core_ids=list(range(N_CORES)), trace=_trace)
    out = merge_outputs(res.results)
    if _trace:
        kernel.last_result = res
    return out
